# revision 21
# baseline (speedup 1.0000x reference)
"""Trainium2 Bass kernel for nn_MultiHeadAttention_8667244003725.

B=4, S=1024, E=1024, H=16, D=64.  Reference:
  q/k/v = einsum('bse,hed->bhsd', x, W{q,k,v})
  scores = q@k^T/sqrt(D), causal mask, softmax
  heads -> concat (B,S,E);  out = W_O @ concat  (contracts over SEQUENCE dim)
  returns (B, E, E).

Sharding: 8 cores = 4 batches x 2 head-groups (8 heads each).  Because the
output projection contracts over the sequence dim, sharding heads shards the
output columns: core c computes out[b, :, 512*g : 512*g+512] with b=c//2,
g=c%2.  No collectives.

v2 layout: software-pipelined so the PE never idles (keeps the HAM clock
gate at 2.4 GHz through the attention phase, which ran at 1.2 GHz in v1):
  Q0,K0 dense -> attention(p) runs with independent filler matmuls
  interleaved into its dependency-wait slots:
    attn(0) <- V-projection MMs,  attn(p) <- Q/K(p+1) MMs,
    attn(3,qc=1) <- junk MMs,  attn(3,qc=0) <- outproj first half (st 4..7).
  Output projection is split in halves (st4..7 accumulated early into an
  SBUF carry ACCP via ACT copies; st0..3 + combine at the tail).
Engine rebalance: causal tri-mask mul on GpSimd (was DVE), per-head-chunk
reciprocals batched [128,4], C normalization as one broadcast
scalar_tensor_tensor per (head, qc) (was 8 DVE ops).
"""

import sys

if '/opt/trn_rl_repo' not in sys.path:
    sys.path.insert(0, '/opt/trn_rl_repo')

from collections import deque

import numpy as np

import concourse.bass as bass
import concourse.mybir as mybir
import concourse.tile as tile
from concourse.masks import make_identity

F32 = mybir.dt.float32
BF = mybir.dt.bfloat16
AF = mybir.ActivationFunctionType
ALU = mybir.AluOpType

S = 1024          # sequence
E = 1024          # embed
D = 64            # head dim
HC = 8            # heads per core
NO = 512          # output columns per core


def _split_sync_waits(nc, limit=1):
    """The walrus build in this env rejects >1 sem-wait per instruction.
    Hoist excess waits onto preceding same-engine no-ops (same queue, so
    program order preserves the wait semantics)."""
    n = 0
    for f in nc.m.functions:
        for bb in f.blocks:
            out = []
            for ins in bb.instructions:
                si = ins.sync_info
                waits = list(si.on_wait) if si is not None else []
                if len(waits) > limit:
                    excess, keep = waits[:-limit], waits[-limit:]
                    for i in range(0, len(excess), limit):
                        grp = excess[i:i + limit]
                        n += 1
                        out.append(mybir.InstNoOp(
                            name=f'I-synsplit-{n}', ins=[], outs=[],
                            engine=ins.engine,
                            sync_info=mybir.SyncInfo(on_wait=list(grp),
                                                     on_update=[])))
                    si.on_wait = keep
                out.append(ins)
            bb.instructions = out
    return n


def build_nc(split_waits=True):
    nc = bass.Bass()
    xb = nc.dram_tensor('xb', [E, S], BF, kind='ExternalInput')   # x[b]^T
    wq = nc.dram_tensor('wq', [E, HC * D], BF, kind='ExternalInput')
    wk = nc.dram_tensor('wk', [E, HC * D], BF, kind='ExternalInput')
    wv = nc.dram_tensor('wv', [E, HC * D], BF, kind='ExternalInput')
    wo = nc.dram_tensor('wo', [E, E], BF, kind='ExternalInput')   # W_O^T
    out = nc.dram_tensor('out', [E, NO], BF, kind='ExternalOutput')

    with tile.TileContext(nc) as tc:
        _emit(nc, tc, xb, wq, wk, wv, wo, out)
    if split_waits:
        _split_sync_waits(nc)
    return nc


def _emit(nc, tc, xb, wq, wk, wv, wo, out):
    with (
        tc.tile_pool(name='const', bufs=1) as constp,
        tc.tile_pool(name='bigT', bufs=2) as bigT,      # xTall + WOTall
        tc.tile_pool(name='wts', bufs=1) as wp,
        tc.tile_pool(name='qk', bufs=1) as qkp,
        tc.tile_pool(name='vall', bufs=1) as vallp,
        tc.tile_pool(name='cbuf', bufs=1) as cp,
        tc.tile_pool(name='accp', bufs=1) as accpp,
        tc.tile_pool(name='attn', bufs=4) as sstr,
        tc.tile_pool(name='ostr', bufs=3) as ostr,
        tc.tile_pool(name='psQK', bufs=2, space='PSUM') as psQK,  # 512 mm
        tc.tile_pool(name='psS', bufs=2, space='PSUM') as psS,    # scores
        tc.tile_pool(name='psOT', bufs=2, space='PSUM') as psOT,  # ot accum
        tc.tile_pool(name='psT', bufs=2, space='PSUM') as psT,    # transposes
    ):
        # ---- PE warm-up: junk matmuls keep the HAM clock gate from
        # idling at 1.2 GHz while the input DMAs trickle in.
        junkt = constp.tile([128, 128], BF, tag='junkt')
        nc.gpsimd.memset(junkt[:], 0.001)
        scrapj = constp.tile([1, 1], F32, tag='scrapj')
        jt = psT.tile([128, 260], F32, tag='tp', name='junkps')
        for _ in range(14):
            nc.tensor.matmul(jt[:, 0:128], junkt[:], junkt[:],
                             start=True, stop=True)

        # ---- constants (gpsimd; must precede the gpsimd DMA triggers) ----
        identf = constp.tile([128, 128], F32, tag='identf')
        make_identity(nc, identf[:])
        ones8 = constp.tile([128, 8], BF, tag='ones8')
        nc.gpsimd.memset(ones8[:], 1.0)
        # multiplicative causal mask for the [128,128] diagonal corner:
        # tri[k, q] = 1 where q >= k else 0
        tri = constp.tile([128, 128], BF, tag='tri')
        nc.gpsimd.memset(tri[:], 1.0)
        nc.gpsimd.affine_select(
            out=tri[:], in_=tri[:], compare_op=ALU.is_ge,
            fill=0.0, base=0, channel_multiplier=-1, pattern=[[1, 128]])

        # ---- input DMA. Trigger instructions cost ~600ns each on the
        # issuing engine and DMAs can only start from SP/ACT/gpsimd, so
        # spread them over three queues and keep the ACT queue nearly
        # clear for the attention exps (in v1 all weight triggers rode
        # the scalar queue and the first exp could not issue until ~31us).
        # sync: xT+wk interleaved (paced for the ec-major Q0/K0
        # consumption) then wo behind; gpsimd: wq; scalar: wv then warm.
        xTall = bigT.tile([128, 8 * S], BF, tag='bigT', name='xTall')
        wqall = wp.tile([128, 8 * HC * D], BF, tag='wqall', name='wqall')
        wkall = wp.tile([128, 8 * HC * D], BF, tag='wkall', name='wkall')
        wvall = wp.tile([128, 8 * HC * D], BF, tag='wvall', name='wvall')
        for ec in range(8):
            nc.sync.dma_start(xTall[:, ec * S:(ec + 1) * S],
                              xb[ec * 128:(ec + 1) * 128, :])
            nc.sync.dma_start(wkall[:, ec * HC * D:(ec + 1) * HC * D],
                              wk[ec * 128:(ec + 1) * 128, :])
            # wq first on both slow queues (first consumer), wv behind
            eng = nc.scalar if ec < 4 else nc.gpsimd
            eng.dma_start(wqall[:, ec * HC * D:(ec + 1) * HC * D],
                          wq[ec * 128:(ec + 1) * 128, :])
        for ec in range(8):
            eng = nc.gpsimd if ec < 4 else nc.scalar
            eng.dma_start(wvall[:, ec * HC * D:(ec + 1) * HC * D],
                          wv[ec * 128:(ec + 1) * 128, :])
        xT = [xTall[:, ec * S:(ec + 1) * S] for ec in range(8)]
        wqt = [wqall[:, ec * HC * D:(ec + 1) * HC * D] for ec in range(8)]
        wkt = [wkall[:, ec * HC * D:(ec + 1) * HC * D] for ec in range(8)]
        wvt = [wvall[:, ec * HC * D:(ec + 1) * HC * D] for ec in range(8)]
        nc.vector.tensor_copy(scrapj[:], jt[0:1, 0:1])  # close junk writes

        # warm the ACT exp table; W_O^T triggers ride the sync queue
        # BEHIND xT+wk (W_O data is not needed until ~85us, and this
        # keeps its 2MB from stealing HBM bandwidth in the lead-in)
        warm = constp.tile([1, 2], F32, tag='warm')
        nc.scalar.activation(warm[:], ones8[0:1, 0:2], AF.Exp, scale=0.125)
        WOTall = bigT.tile([128, 8 * E], BF, tag='bigT', name='WOTall')
        for sc in range(8):
            nc.sync.dma_start(WOTall[:, sc * E:(sc + 1) * E],
                              wo[sc * 128:(sc + 1) * 128, :])
        WOT = [WOTall[:, sc * E:(sc + 1) * E] for sc in range(8)]

        # ---- SBUF destinations -------------------------------------------
        QT2 = [qkp.tile([128, S], BF, tag=f'q{p}', name=f'QT2_{p}')
               for p in range(4)]
        KT2 = [qkp.tile([128, S], BF, tag=f'k{p}', name=f'KT2_{p}')
               for p in range(4)]
        Vall = [vallp.tile([128, HC * (D + 1)], BF, tag=f'v{st}',
                           name=f'Vall{st}') for st in range(8)]
        # C split: pairs 0..2 (cols 0:384 of each st chunk) vs pair 3
        # (cols 384:512), so the output projection over pair-0..2 columns
        # can run as PE filler inside attn(3) without read/write overlap
        # on the tile still being written by pair 3.
        Cmain = cp.tile([128, 8 * 384], BF, tag='cmain', name='Cmain')
        cm3 = Cmain[:].rearrange('p (st c) -> p st c', c=384)
        Cp3 = cp.tile([128, 8 * 128], BF, tag='cp3', name='Cp3')
        cp3_ = Cp3[:].rearrange('p (st c) -> p st c', c=128)
        # assembled output rows (bf16) awaiting DMA
        Ybig = accpp.tile([128, 8 * NO], BF, tag='ybig', name='Ybig')

        # ---- filler infrastructure ---------------------------------------
        # Units are (gid, fn) closures emitting a couple of independent PE
        # matmuls; attention emission drains them into its dependency-wait
        # slots so the PE queue never runs dry (keeps HAM at K=8/8).
        units = deque()
        junk_on = [False]
        junk_pool = [None]

        def emit_junk():
            # fresh tile each call: the pool's WAR/WAW tracking keeps the
            # junk writes ordered against that buffer's previous users.
            # attn(3, qc=1) borrows the idle psQK rotation; the tail uses
            # psS (psQK may still have in-flight ACT readers there).
            pool, tag = junk_pool[0]
            jq = pool.tile([128, 512], F32, tag=tag, name='junkfill')
            nc.tensor.matmul(jq[:, 0:128], junkt[:], junkt[:],
                             start=True, stop=True)
            nc.tensor.matmul(jq[:, 128:256], junkt[:], junkt[:],
                             start=True, stop=True)

        def fill(n):
            for _ in range(n):
                if units:
                    units.popleft()[1]()
                elif junk_on[0]:
                    emit_junk()

        def drain_upto(gid):
            while units and units[0][0] <= gid:
                units.popleft()[1]()

        def drain_all():
            while units:
                units.popleft()[1]()

        # ---- QK jobs ------------------------------------------------------
        def qk_units(p):
            """Units computing QT2[p] / KT2[p] (each: 2 accumulation MMs or
            the psum->sbuf cast)."""
            us = []
            for dst, wt in ((QT2[p], wqt), (KT2[p], wkt)):
                for sc in range(2):
                    box = {}

                    def mk_mm(ec0, dst=dst, wt=wt, sc=sc, box=box):
                        def go():
                            if 'ps' not in box:
                                box['ps'] = psQK.tile([128, 512], F32,
                                                      tag='qk', name='qkps')
                            for ec in (ec0, ec0 + 1):
                                nc.tensor.matmul(
                                    box['ps'][:],
                                    wt[ec][:, p * 128:(p + 1) * 128],
                                    xT[ec][:, sc * 512:(sc + 1) * 512],
                                    start=(ec == 0), stop=(ec == 7))
                        return go

                    def mk_cp(dst=dst, sc=sc, box=box):
                        def go():
                            nc.vector.tensor_copy(
                                dst[:, sc * 512:(sc + 1) * 512],
                                box['ps'][:])
                        return go

                    for ec0 in range(0, 8, 2):
                        us.append((100 + p, mk_mm(ec0)))
                    us.append((100 + p, mk_cp()))
            return us

        def v_units():
            """Units computing Vall[st] (value proj + ones column)."""
            us = []
            for st in range(8):
                box = {}

                def mk_mm(ec0, st=st, box=box):
                    def go():
                        if 'ps' not in box:
                            box['ps'] = psQK.tile([128, 512], F32, tag='qk',
                                                  name='vps')
                        for ec in (ec0, ec0 + 1):
                            nc.tensor.matmul(
                                box['ps'][:],
                                xT[ec][:, st * 128:(st + 1) * 128],
                                wvt[ec],
                                start=(ec == 0), stop=(ec == 7))
                    return go

                def mk_cp(st=st, box=box):
                    def go():
                        v3 = Vall[st][:].rearrange('p (h d) -> p h d', h=HC)
                        nc.vector.tensor_copy(
                            v3[:, :, 0:D],
                            box['ps'][:].rearrange('p (h d) -> p h d', h=HC))
                        nc.vector.tensor_copy(
                            v3[:, :, D:D + 1],
                            ones8[:].rearrange('p (h o) -> p h o', o=1))
                    return go

                for ec0 in range(0, 8, 2):
                    us.append((st, mk_mm(ec0)))
                us.append((st, mk_cp()))
            return us

        def outproj_main_units():
            """out[it][:, 0:384] (pair 0..2 columns) over ALL st chunks;
            legal as soon as attn(2) is done -> PE filler for attn(3)."""
            us = []
            for it in range(8):
                box = {}

                def mk_mm(st0, it=it, box=box):
                    def go():
                        if 'ps' not in box:
                            box['ps'] = psQK.tile([128, 384], F32, tag='qk',
                                                  name=f'opm_{it}')
                        for st in (st0, st0 + 1):
                            nc.tensor.matmul(
                                box['ps'][:],
                                WOT[st][:, it * 128:(it + 1) * 128],
                                Cmain[:, st * 384:(st + 1) * 384],
                                start=(st == 0), stop=(st == 7))
                    return go

                def mk_fin(it=it, box=box):
                    def go():
                        nc.scalar.copy(Ybig[:, it * NO:it * NO + 384],
                                       box['ps'][:])
                    return go

                for st0 in range(0, 8, 2):
                    us.append((200 + it, mk_mm(st0)))
                us.append((200 + it, mk_fin()))
            return us

        def outproj_p3_units():
            """out[it][:, 384:512] (pair 3 columns) + row DMA; needs all
            of attn(3) done -> tail (after a short junk bridge)."""
            us = []
            for it in range(8):
                box = {}

                def mk_mm(st0, it=it, box=box):
                    def go():
                        if 'ps' not in box:
                            box['ps'] = psS.tile([128, 128], F32, tag='s',
                                                 name=f'opp_{it}')
                        for st in range(st0, st0 + 4):
                            nc.tensor.matmul(
                                box['ps'][:],
                                WOT[st][:, it * 128:(it + 1) * 128],
                                Cp3[:, st * 128:(st + 1) * 128],
                                start=(st == 0), stop=(st == 7))
                    return go

                def mk_fin(it=it, box=box):
                    def go():
                        nc.vector.tensor_copy(
                            Ybig[:, it * NO + 384:(it + 1) * NO],
                            box['ps'][:])
                        nc.sync.dma_start(out[it * 128:(it + 1) * 128, :],
                                          Ybig[:, it * NO:(it + 1) * NO])
                    return go

                us.append((220 + it, mk_mm(0)))
                us.append((220 + it, mk_mm(4)))
                us.append((220 + it, mk_fin()))
            return us

        # ---- attention ----------------------------------------------------
        def attention(p, qc):
            heads = (2 * p, 2 * p + 1)
            QTh = {h: QT2[p][64 * (h % 2):64 * (h % 2) + 64, :]
                   for h in heads}
            KTh = {h: KT2[p][64 * (h % 2):64 * (h % 2) + 64, :]
                   for h in heads}
            nkb = 8 if qc == 1 else 4
            ots = {h: psOT.tile([128, 512], F32, tag='ot',
                                name=f'ot_{h}_{qc}') for h in heads}
            for kb in range(nkb):
                j = kb - 4 * qc
                off = 128 * j if j >= 0 else 0
                W = 512 - off
                is_diag = j >= 0
                if p == 0:
                    drain_upto(kb)   # V(st<=kb) must precede OT(kb)
                sps, pex = {}, {}
                for h in heads:
                    sps[h] = psS.tile([128, 512], F32, tag='s',
                                      name=f's_{h}_{qc}_{kb}')
                    nc.tensor.matmul(
                        sps[h][:, 0:W],
                        KTh[h][:, kb * 128:(kb + 1) * 128],
                        QTh[h][:, qc * 512 + off:(qc + 1) * 512],
                        start=True, stop=True)
                fill(1)
                for h in heads:
                    pex[h] = sstr.tile([128, 512], BF, tag='pexp',
                                       name=f'pex_{h}_{qc}_{kb}')
                    nc.scalar.activation(pex[h][:, 0:W], sps[h][:, 0:W],
                                         AF.Exp, scale=0.125)
                    if is_diag:
                        nc.gpsimd.tensor_mul(pex[h][:, 0:128],
                                             pex[h][:, 0:128], tri[:])
                fill(1)
                for h in heads:
                    nc.tensor.matmul(
                        ots[h][:D + 1, off:512],
                        Vall[kb][:, h * (D + 1):(h + 1) * (D + 1)],
                        pex[h][:, 0:W],
                        start=(kb == 0), stop=(kb == nkb - 1))
                fill(2)
            # transpose [65,512] back to q-major, normalize by l, write C
            for h in heads:
                osb = sstr.tile([D + 1, 512], F32, tag='ots')
                nc.vector.tensor_copy(osb[:], ots[h][:D + 1, :])
                tpT = psT.tile([128, 4 * (D + 1)], F32, tag='tp')
                for qb in range(4):
                    nc.tensor.transpose(
                        tpT[:, qb * 65:qb * 65 + 65],
                        osb[:, qb * 128:(qb + 1) * 128],
                        identf[:D + 1, :D + 1])
                fill(1)
                tp3 = tpT[:].rearrange('p (a c) -> p a c', c=D + 1)
                rl = sstr.tile([128, 4], F32, tag='rl')
                rl3 = rl[:].rearrange('p (a c) -> p a c', c=1)
                nc.vector.reciprocal(rl3[:, :, :], tp3[:, :, D:D + 1])
                if p < 3:
                    cdst = cm3[:, 4 * qc:4 * qc + 4, h * D:(h + 1) * D]
                else:
                    cdst = cp3_[:, 4 * qc:4 * qc + 4,
                                (h - 6) * D:(h - 5) * D]
                nc.vector.scalar_tensor_tensor(
                    out=cdst,
                    in0=tp3[:, :, 0:D],
                    scalar=1.0,
                    in1=rl3.broadcast_to([128, 4, D]),
                    op0=ALU.mult, op1=ALU.mult)
                fill(1)

        # ---- emission schedule -------------------------------------------
        # Q0, K0: DMA-paced (the lead-in is HBM-bandwidth-bound), so pad
        # with junk matmuls between units to keep the HAM activity window
        # busy while the PE sits on input-arrival semaphores.
        junk_pool[0] = (psS, 's')
        for _, fn in qk_units(0):
            fn()
            emit_junk()
        # V and QK(1) both ride the DMA-bound lead-in window / attn(0)
        units.extend(v_units())
        units.extend(qk_units(1))
        for p in range(4):
            if p < 3:
                # QK(p+1) fills attn(p)'s wait slots (gids 100+: never
                # pulled in by drain_upto, which only targets V units)
                if p > 0:
                    units.extend(qk_units(p + 1))
                attention(p, 1)   # qc=1 first: C[4..7] complete sooner
                attention(p, 0)
                drain_all()
            else:
                # outproj over pair-0..2 columns is ready now and fills
                # attn(3)'s wait slots; junk as a backstop if it runs dry
                units.extend(outproj_main_units())
                junk_pool[0] = (psQK, 'qk')
                junk_on[0] = True
                attention(3, 1)
                attention(3, 0)
                drain_all()
                junk_on[0] = False
        # tail: pair-3 output columns + row DMAs.  A few junk matmuls
        # cover the DVE latency of the last C writes so the PE does not
        # idle (and HAM-throttle) right before the final burst.
        junk_pool[0] = (psS, 's')
        for _ in range(4):
            emit_junk()
        for _, fn in outproj_p3_units():
            fn()


_NC_CACHE = None


def _get_nc():
    global _NC_CACHE
    if _NC_CACHE is None:
        _NC_CACHE = build_nc()
    return _NC_CACHE


def make_in_maps(x, Wq, Wk, Wv, W_O):
    import ml_dtypes
    bf = ml_dtypes.bfloat16
    x = np.asarray(x, np.float32)
    xT_by_b = [np.ascontiguousarray(x[b].T.astype(bf)) for b in range(4)]
    W_O = np.ascontiguousarray(np.asarray(W_O, np.float32).T.astype(bf))
    in_maps = []
    for c in range(8):
        b, g = c // 2, c % 2
        hsl = slice(HC * g, HC * g + HC)
        in_maps.append({
            'xb': xT_by_b[b],
            'wq': np.ascontiguousarray(
                np.asarray(Wq, np.float32)[hsl].transpose(1, 0, 2)
                .reshape(E, HC * D).astype(bf)),
            'wk': np.ascontiguousarray(
                np.asarray(Wk, np.float32)[hsl].transpose(1, 0, 2)
                .reshape(E, HC * D).astype(bf)),
            'wv': np.ascontiguousarray(
                np.asarray(Wv, np.float32)[hsl].transpose(1, 0, 2)
                .reshape(E, HC * D).astype(bf)),
            'wo': W_O,
        })
    return in_maps


def kernel(x, Wq, Wk, Wv, W_O):
    from concourse.bass_utils import run_bass_kernel_spmd
    nc = _get_nc()
    in_maps = make_in_maps(x, Wq, Wk, Wv, W_O)
    res = run_bass_kernel_spmd(nc, in_maps, list(range(8)))
    full = np.empty((4, E, E), np.float32)
    for c in range(8):
        b, g = c // 2, c % 2
        full[b, :, NO * g:NO * g + NO] = res.results[c]['out']
    return full


# revision 24
# speedup vs baseline: 1.0062x; 1.0062x over previous
"""Trainium2 Bass kernel for nn_MultiHeadAttention_8667244003725.

B=4, S=1024, E=1024, H=16, D=64.  Reference:
  q/k/v = einsum('bse,hed->bhsd', x, W{q,k,v})
  scores = q@k^T/sqrt(D), causal mask, softmax
  heads -> concat (B,S,E);  out = W_O @ concat  (contracts over SEQUENCE dim)
  returns (B, E, E).

Sharding: 8 cores = 4 batches x 2 head-groups (8 heads each).  Because the
output projection contracts over the sequence dim, sharding heads shards the
output columns: core c computes out[b, :, 512*g : 512*g+512] with b=c//2,
g=c%2.  No collectives.

v2 layout: software-pipelined so the PE never idles (keeps the HAM clock
gate at 2.4 GHz through the attention phase, which ran at 1.2 GHz in v1):
  Q0,K0 dense -> attention(p) runs with independent filler matmuls
  interleaved into its dependency-wait slots:
    attn(0) <- V-projection MMs,  attn(p) <- Q/K(p+1) MMs,
    attn(3,qc=1) <- junk MMs,  attn(3,qc=0) <- outproj first half (st 4..7).
  Output projection is split in halves (st4..7 accumulated early into an
  SBUF carry ACCP via ACT copies; st0..3 + combine at the tail).
Engine rebalance: causal tri-mask mul on GpSimd (was DVE), per-head-chunk
reciprocals batched [128,4], C normalization as one broadcast
scalar_tensor_tensor per (head, qc) (was 8 DVE ops).
"""

import sys

if '/opt/trn_rl_repo' not in sys.path:
    sys.path.insert(0, '/opt/trn_rl_repo')

from collections import deque

import numpy as np

import concourse.bass as bass
import concourse.mybir as mybir
import concourse.tile as tile
from concourse.masks import make_identity

F32 = mybir.dt.float32
BF = mybir.dt.bfloat16
AF = mybir.ActivationFunctionType
ALU = mybir.AluOpType

S = 1024          # sequence
E = 1024          # embed
D = 64            # head dim
HC = 8            # heads per core
NO = 512          # output columns per core


def _split_sync_waits(nc, limit=1):
    """The walrus build in this env rejects >1 sem-wait per instruction.
    Hoist excess waits onto preceding same-engine no-ops (same queue, so
    program order preserves the wait semantics)."""
    n = 0
    for f in nc.m.functions:
        for bb in f.blocks:
            out = []
            for ins in bb.instructions:
                si = ins.sync_info
                waits = list(si.on_wait) if si is not None else []
                if len(waits) > limit:
                    excess, keep = waits[:-limit], waits[-limit:]
                    for i in range(0, len(excess), limit):
                        grp = excess[i:i + limit]
                        n += 1
                        out.append(mybir.InstNoOp(
                            name=f'I-synsplit-{n}', ins=[], outs=[],
                            engine=ins.engine,
                            sync_info=mybir.SyncInfo(on_wait=list(grp),
                                                     on_update=[])))
                    si.on_wait = keep
                out.append(ins)
            bb.instructions = out
    return n


def build_nc(split_waits=True):
    nc = bass.Bass()
    xb = nc.dram_tensor('xb', [E, S], BF, kind='ExternalInput')   # x[b]^T
    wq = nc.dram_tensor('wq', [E, HC * D], BF, kind='ExternalInput')
    wk = nc.dram_tensor('wk', [E, HC * D], BF, kind='ExternalInput')
    wv = nc.dram_tensor('wv', [E, HC * D], BF, kind='ExternalInput')
    wo = nc.dram_tensor('wo', [E, E], BF, kind='ExternalInput')   # W_O^T
    out = nc.dram_tensor('out', [E, NO], BF, kind='ExternalOutput')

    with tile.TileContext(nc) as tc:
        _emit(nc, tc, xb, wq, wk, wv, wo, out)
    if split_waits:
        _split_sync_waits(nc)
    return nc


def _emit(nc, tc, xb, wq, wk, wv, wo, out):
    with (
        tc.tile_pool(name='const', bufs=1) as constp,
        tc.tile_pool(name='bigT', bufs=2) as bigT,      # xTall + WOTall
        tc.tile_pool(name='wts', bufs=1) as wp,
        tc.tile_pool(name='qk', bufs=1) as qkp,
        tc.tile_pool(name='vall', bufs=1) as vallp,
        tc.tile_pool(name='cbuf', bufs=1) as cp,
        tc.tile_pool(name='accp', bufs=1) as accpp,
        tc.tile_pool(name='attn', bufs=4) as sstr,
        tc.tile_pool(name='ostr', bufs=3) as ostr,
        tc.tile_pool(name='psQK', bufs=2, space='PSUM') as psQK,  # 512 mm
        tc.tile_pool(name='psS', bufs=2, space='PSUM') as psS,    # scores
        tc.tile_pool(name='psOT', bufs=2, space='PSUM') as psOT,  # ot accum
        tc.tile_pool(name='psT', bufs=2, space='PSUM') as psT,    # transposes
    ):
        # ---- PE warm-up: junk matmuls keep the HAM clock gate from
        # idling at 1.2 GHz while the input DMAs trickle in.
        junkt = constp.tile([128, 128], BF, tag='junkt')
        nc.gpsimd.memset(junkt[:], 0.001)
        scrapj = constp.tile([1, 1], F32, tag='scrapj')
        jt = psT.tile([128, 260], F32, tag='tp', name='junkps')
        for _ in range(14):
            nc.tensor.matmul(jt[:, 0:128], junkt[:], junkt[:],
                             start=True, stop=True)

        # ---- constants (gpsimd; must precede the gpsimd DMA triggers) ----
        identf = constp.tile([128, 128], F32, tag='identf')
        make_identity(nc, identf[:])
        ones8 = constp.tile([128, 8], BF, tag='ones8')
        nc.gpsimd.memset(ones8[:], 1.0)
        # multiplicative causal mask for the [128,128] diagonal corner:
        # tri[k, q] = 1 where q >= k else 0
        tri = constp.tile([128, 128], BF, tag='tri')
        nc.gpsimd.memset(tri[:], 1.0)
        nc.gpsimd.affine_select(
            out=tri[:], in_=tri[:], compare_op=ALU.is_ge,
            fill=0.0, base=0, channel_multiplier=-1, pattern=[[1, 128]])

        # ---- input DMA. Trigger instructions cost ~600ns each on the
        # issuing engine and DMAs can only start from SP/ACT/gpsimd, so
        # spread them over three queues and keep the ACT queue nearly
        # clear for the attention exps (in v1 all weight triggers rode
        # the scalar queue and the first exp could not issue until ~31us).
        # sync: xT+wk interleaved (paced for the ec-major Q0/K0
        # consumption) then wo behind; gpsimd: wq; scalar: wv then warm.
        xTall = bigT.tile([128, 8 * S], BF, tag='bigT', name='xTall')
        wqall = wp.tile([128, 8 * HC * D], BF, tag='wqall', name='wqall')
        wkall = wp.tile([128, 8 * HC * D], BF, tag='wkall', name='wkall')
        wvall = wp.tile([128, 8 * HC * D], BF, tag='wvall', name='wvall')
        for ec in range(8):
            nc.sync.dma_start(xTall[:, ec * S:(ec + 1) * S],
                              xb[ec * 128:(ec + 1) * 128, :])
            nc.sync.dma_start(wkall[:, ec * HC * D:(ec + 1) * HC * D],
                              wk[ec * 128:(ec + 1) * 128, :])
            # wq first on both slow queues (first consumer), wv behind
            eng = nc.scalar if ec < 4 else nc.gpsimd
            eng.dma_start(wqall[:, ec * HC * D:(ec + 1) * HC * D],
                          wq[ec * 128:(ec + 1) * 128, :])
        for ec in range(8):
            eng = nc.gpsimd if ec < 4 else nc.scalar
            eng.dma_start(wvall[:, ec * HC * D:(ec + 1) * HC * D],
                          wv[ec * 128:(ec + 1) * 128, :])
        xT = [xTall[:, ec * S:(ec + 1) * S] for ec in range(8)]
        wqt = [wqall[:, ec * HC * D:(ec + 1) * HC * D] for ec in range(8)]
        wkt = [wkall[:, ec * HC * D:(ec + 1) * HC * D] for ec in range(8)]
        wvt = [wvall[:, ec * HC * D:(ec + 1) * HC * D] for ec in range(8)]
        nc.vector.tensor_copy(scrapj[:], jt[0:1, 0:1])  # close junk writes

        # warm the ACT exp table; W_O^T triggers ride the sync queue
        # BEHIND xT+wk (W_O data is not needed until ~85us, and this
        # keeps its 2MB from stealing HBM bandwidth in the lead-in)
        warm = constp.tile([1, 2], F32, tag='warm')
        nc.scalar.activation(warm[:], ones8[0:1, 0:2], AF.Exp, scale=0.125)
        WOTall = bigT.tile([128, 8 * E], BF, tag='bigT', name='WOTall')
        for sc in range(8):
            nc.sync.dma_start(WOTall[:, sc * E:(sc + 1) * E],
                              wo[sc * 128:(sc + 1) * 128, :])
        WOT = [WOTall[:, sc * E:(sc + 1) * E] for sc in range(8)]

        # ---- SBUF destinations -------------------------------------------
        QT2 = [qkp.tile([128, S], BF, tag=f'q{p}', name=f'QT2_{p}')
               for p in range(4)]
        KT2 = [qkp.tile([128, S], BF, tag=f'k{p}', name=f'KT2_{p}')
               for p in range(4)]
        Vall = [vallp.tile([128, HC * (D + 1)], BF, tag=f'v{st}',
                           name=f'Vall{st}') for st in range(8)]
        # C split: pairs 0..2 (cols 0:384 of each st chunk) vs pair 3
        # (cols 384:512), so the output projection over pair-0..2 columns
        # can run as PE filler inside attn(3) without read/write overlap
        # on the tile still being written by pair 3.
        Cmain = cp.tile([128, 8 * 384], BF, tag='cmain', name='Cmain')
        cm3 = Cmain[:].rearrange('p (st c) -> p st c', c=384)
        Cp3 = cp.tile([128, 8 * 128], BF, tag='cp3', name='Cp3')
        cp3_ = Cp3[:].rearrange('p (st c) -> p st c', c=128)
        # assembled output rows (bf16) awaiting DMA
        Ybig = accpp.tile([128, 8 * NO], BF, tag='ybig', name='Ybig')

        # ---- filler infrastructure ---------------------------------------
        # Units are (gid, fn) closures emitting a couple of independent PE
        # matmuls; attention emission drains them into its dependency-wait
        # slots so the PE queue never runs dry (keeps HAM at K=8/8).
        units = deque()
        junk_on = [False]
        junk_pool = [None]

        def emit_junk():
            # fresh tile each call: the pool's WAR/WAW tracking keeps the
            # junk writes ordered against that buffer's previous users.
            # attn(3, qc=1) borrows the idle psQK rotation; the tail uses
            # psS (psQK may still have in-flight ACT readers there).
            pool, tag = junk_pool[0]
            jq = pool.tile([128, 512], F32, tag=tag, name='junkfill')
            nc.tensor.matmul(jq[:, 0:128], junkt[:], junkt[:],
                             start=True, stop=True)
            nc.tensor.matmul(jq[:, 128:256], junkt[:], junkt[:],
                             start=True, stop=True)

        def fill(n):
            for _ in range(n):
                if units:
                    units.popleft()[1]()
                elif junk_on[0]:
                    emit_junk()

        def drain_upto(gid):
            while units and units[0][0] <= gid:
                units.popleft()[1]()

        def drain_all():
            while units:
                units.popleft()[1]()

        # ---- QK jobs ------------------------------------------------------
        def qk_units(p):
            """Units computing QT2[p] / KT2[p] (each: 2 accumulation MMs or
            the psum->sbuf cast)."""
            us = []
            for dst, wt in ((QT2[p], wqt), (KT2[p], wkt)):
                for sc in range(2):
                    box = {}

                    def mk_mm(ec0, dst=dst, wt=wt, sc=sc, box=box):
                        def go():
                            if 'ps' not in box:
                                box['ps'] = psQK.tile([128, 512], F32,
                                                      tag='qk', name='qkps')
                            for ec in (ec0, ec0 + 1):
                                nc.tensor.matmul(
                                    box['ps'][:],
                                    wt[ec][:, p * 128:(p + 1) * 128],
                                    xT[ec][:, sc * 512:(sc + 1) * 512],
                                    start=(ec == 0), stop=(ec == 7))
                        return go

                    def mk_cp(dst=dst, sc=sc, box=box):
                        def go():
                            nc.vector.tensor_copy(
                                dst[:, sc * 512:(sc + 1) * 512],
                                box['ps'][:])
                        return go

                    for ec0 in range(0, 8, 2):
                        us.append((100 + p, mk_mm(ec0)))
                    us.append((100 + p, mk_cp()))
            return us

        def v_units():
            """Units computing Vall[st] (value proj + ones column)."""
            us = []
            for st in range(8):
                box = {}

                def mk_mm(ec0, st=st, box=box):
                    def go():
                        if 'ps' not in box:
                            box['ps'] = psQK.tile([128, 512], F32, tag='qk',
                                                  name='vps')
                        for ec in (ec0, ec0 + 1):
                            nc.tensor.matmul(
                                box['ps'][:],
                                xT[ec][:, st * 128:(st + 1) * 128],
                                wvt[ec],
                                start=(ec == 0), stop=(ec == 7))
                    return go

                def mk_cp(st=st, box=box):
                    def go():
                        v3 = Vall[st][:].rearrange('p (h d) -> p h d', h=HC)
                        nc.vector.tensor_copy(
                            v3[:, :, 0:D],
                            box['ps'][:].rearrange('p (h d) -> p h d', h=HC))
                        nc.vector.tensor_copy(
                            v3[:, :, D:D + 1],
                            ones8[:].rearrange('p (h o) -> p h o', o=1))
                    return go

                for ec0 in range(0, 8, 2):
                    us.append((st, mk_mm(ec0)))
                us.append((st, mk_cp()))
            return us

        def outproj_main_units():
            """out[it][:, 0:384] (pair 0..2 columns) over ALL st chunks;
            legal as soon as attn(2) is done -> PE filler for attn(3)."""
            us = []
            for it in range(8):
                box = {}

                def mk_mm(st0, it=it, box=box):
                    def go():
                        if 'ps' not in box:
                            box['ps'] = psQK.tile([128, 384], F32, tag='qk',
                                                  name=f'opm_{it}')
                        for st in (st0, st0 + 1):
                            nc.tensor.matmul(
                                box['ps'][:],
                                WOT[st][:, it * 128:(it + 1) * 128],
                                Cmain[:, st * 384:(st + 1) * 384],
                                start=(st == 0), stop=(st == 7))
                    return go

                def mk_fin(it=it, box=box):
                    def go():
                        # DVE, not ACT: a fin waiting on its (filler) MMs
                        # in the strict ACT FIFO would block later exps
                        # and stall the attention chain behind them
                        nc.vector.tensor_copy(
                            Ybig[:, it * NO:it * NO + 384], box['ps'][:])
                    return go

                for st0 in range(0, 8, 2):
                    us.append((200 + it, mk_mm(st0)))
                us.append((200 + it, mk_fin()))
            return us

        def outproj_p3_units():
            """out[it][:, 384:512] (pair 3 columns) + row DMA; needs all
            of attn(3) done -> tail (after a short junk bridge)."""
            us = []
            for it in range(8):
                box = {}

                def mk_mm(st0, it=it, box=box):
                    def go():
                        if 'ps' not in box:
                            box['ps'] = psS.tile([128, 128], F32, tag='s',
                                                 name=f'opp_{it}')
                        for st in range(st0, st0 + 4):
                            nc.tensor.matmul(
                                box['ps'][:],
                                WOT[st][:, it * 128:(it + 1) * 128],
                                Cp3[:, st * 128:(st + 1) * 128],
                                start=(st == 0), stop=(st == 7))
                    return go

                def mk_fin(it=it, box=box):
                    def go():
                        nc.vector.tensor_copy(
                            Ybig[:, it * NO + 384:(it + 1) * NO],
                            box['ps'][:])
                        nc.sync.dma_start(out[it * 128:(it + 1) * 128, :],
                                          Ybig[:, it * NO:(it + 1) * NO])
                    return go

                us.append((220 + it, mk_mm(0)))
                us.append((220 + it, mk_mm(4)))
                us.append((220 + it, mk_fin()))
            return us

        # ---- attention ----------------------------------------------------
        def attention(p, qc):
            heads = (2 * p, 2 * p + 1)
            QTh = {h: QT2[p][64 * (h % 2):64 * (h % 2) + 64, :]
                   for h in heads}
            KTh = {h: KT2[p][64 * (h % 2):64 * (h % 2) + 64, :]
                   for h in heads}
            nkb = 8 if qc == 1 else 4
            ots = {h: psOT.tile([128, 512], F32, tag='ot',
                                name=f'ot_{h}_{qc}') for h in heads}
            for kb in range(nkb):
                j = kb - 4 * qc
                off = 128 * j if j >= 0 else 0
                W = 512 - off
                is_diag = j >= 0
                if p == 0:
                    drain_upto(kb)   # V(st<=kb) must precede OT(kb)
                sps, pex = {}, {}
                for h in heads:
                    sps[h] = psS.tile([128, 512], F32, tag='s',
                                      name=f's_{h}_{qc}_{kb}')
                    nc.tensor.matmul(
                        sps[h][:, 0:W],
                        KTh[h][:, kb * 128:(kb + 1) * 128],
                        QTh[h][:, qc * 512 + off:(qc + 1) * 512],
                        start=True, stop=True)
                fill(1)
                for h in heads:
                    pex[h] = sstr.tile([128, 512], BF, tag='pexp',
                                       name=f'pex_{h}_{qc}_{kb}')
                    nc.scalar.activation(pex[h][:, 0:W], sps[h][:, 0:W],
                                         AF.Exp, scale=0.125)
                    if is_diag:
                        nc.gpsimd.tensor_mul(pex[h][:, 0:128],
                                             pex[h][:, 0:128], tri[:])
                fill(1)
                for h in heads:
                    nc.tensor.matmul(
                        ots[h][:D + 1, off:512],
                        Vall[kb][:, h * (D + 1):(h + 1) * (D + 1)],
                        pex[h][:, 0:W],
                        start=(kb == 0), stop=(kb == nkb - 1))
                fill(2)
            fill(2)   # cover the OT->DVE-copy->transpose latency
            # transpose [65,512] back to q-major, normalize by l, write C
            for h in heads:
                osb = sstr.tile([D + 1, 512], F32, tag='ots')
                nc.vector.tensor_copy(osb[:], ots[h][:D + 1, :])
                tpT = psT.tile([128, 4 * (D + 1)], F32, tag='tp')
                for qb in range(4):
                    nc.tensor.transpose(
                        tpT[:, qb * 65:qb * 65 + 65],
                        osb[:, qb * 128:(qb + 1) * 128],
                        identf[:D + 1, :D + 1])
                fill(1)
                tp3 = tpT[:].rearrange('p (a c) -> p a c', c=D + 1)
                rl = sstr.tile([128, 4], F32, tag='rl')
                rl3 = rl[:].rearrange('p (a c) -> p a c', c=1)
                nc.vector.reciprocal(rl3[:, :, :], tp3[:, :, D:D + 1])
                if p < 3:
                    cdst = cm3[:, 4 * qc:4 * qc + 4, h * D:(h + 1) * D]
                else:
                    cdst = cp3_[:, 4 * qc:4 * qc + 4,
                                (h - 6) * D:(h - 5) * D]
                nc.vector.scalar_tensor_tensor(
                    out=cdst,
                    in0=tp3[:, :, 0:D],
                    scalar=1.0,
                    in1=rl3.broadcast_to([128, 4, D]),
                    op0=ALU.mult, op1=ALU.mult)
                fill(1)

        # ---- emission schedule -------------------------------------------
        # Q0, K0 direct (the lead-in is HBM-bandwidth-bound; the PE just
        # follows input arrival there)
        for _, fn in qk_units(0):
            fn()
        # V rides the rest of the lead-in window / attn(0).  Filler for
        # attn(p) is QK(p+1), split so attn(p, qc=0) is not starved after
        # a greedy qc=1 (gids 100+: never pulled in by drain_upto, which
        # only targets V units).
        units.extend(v_units())
        for p in range(4):
            if p < 3:
                nxt = qk_units(p + 1)
                units.extend(nxt[:12])
                attention(p, 1)   # qc=1 first: C[4..7] complete sooner
                units.extend(nxt[12:])
                attention(p, 0)
                drain_all()
            else:
                # outproj over pair-0..2 columns is ready now and fills
                # attn(3)'s wait slots; junk as a backstop if it runs dry
                opm = outproj_main_units()
                units.extend(opm[:25])
                junk_pool[0] = (psQK, 'qk')
                junk_on[0] = True
                attention(3, 1)
                units.extend(opm[25:])
                attention(3, 0)
                drain_all()
                junk_on[0] = False
        # tail: pair-3 output columns + row DMAs.  A few junk matmuls
        # cover the DVE latency of the last C writes so the PE does not
        # idle (and HAM-throttle) right before the final burst.
        junk_pool[0] = (psS, 's')
        for _ in range(4):
            emit_junk()
        for _, fn in outproj_p3_units():
            fn()


_NC_CACHE = None


def _get_nc():
    global _NC_CACHE
    if _NC_CACHE is None:
        _NC_CACHE = build_nc()
    return _NC_CACHE


def make_in_maps(x, Wq, Wk, Wv, W_O):
    import ml_dtypes
    bf = ml_dtypes.bfloat16
    x = np.asarray(x, np.float32)
    xT_by_b = [np.ascontiguousarray(x[b].T.astype(bf)) for b in range(4)]
    W_O = np.ascontiguousarray(np.asarray(W_O, np.float32).T.astype(bf))
    in_maps = []
    for c in range(8):
        b, g = c // 2, c % 2
        hsl = slice(HC * g, HC * g + HC)
        in_maps.append({
            'xb': xT_by_b[b],
            'wq': np.ascontiguousarray(
                np.asarray(Wq, np.float32)[hsl].transpose(1, 0, 2)
                .reshape(E, HC * D).astype(bf)),
            'wk': np.ascontiguousarray(
                np.asarray(Wk, np.float32)[hsl].transpose(1, 0, 2)
                .reshape(E, HC * D).astype(bf)),
            'wv': np.ascontiguousarray(
                np.asarray(Wv, np.float32)[hsl].transpose(1, 0, 2)
                .reshape(E, HC * D).astype(bf)),
            'wo': W_O,
        })
    return in_maps


def kernel(x, Wq, Wk, Wv, W_O):
    from concourse.bass_utils import run_bass_kernel_spmd
    nc = _get_nc()
    in_maps = make_in_maps(x, Wq, Wk, Wv, W_O)
    res = run_bass_kernel_spmd(nc, in_maps, list(range(8)))
    full = np.empty((4, E, E), np.float32)
    for c in range(8):
        b, g = c // 2, c % 2
        full[b, :, NO * g:NO * g + NO] = res.results[c]['out']
    return full
